# revision 4
# baseline (speedup 1.0000x reference)
"""Trainium2 Bass kernel for the CustomJacobiLayer problem.

Computes out[b,j] = sum_{i,d} P_d(tanh(x[b,i])) * coef[j,i,d]
with P_d the Jacobi(alpha=1,beta=1) polynomials, d=0..7.

Strategy (8 NeuronCores, data-parallel over batch):
  - Each core owns 512 of the 4096 batch rows; coef is replicated.
  - Host-side: the three-term Jacobi recurrence
        p_d = K1_d * t * p_{d-1} - K3_d * p_{d-2}     (K2_d == 0 for a==b)
    is rescaled with q_d = p_d / s_d, s_d = K1_d * s_{d-1}, so the device
    recurrence has a unit leading coefficient:
        q_d = t * q_{d-1} - g_d * q_{d-2}
    The scales s_d are folded into coef (in float64), so only two fp16
    VectorE ops per order are needed on-chip.
  - The d=0 term is P_0 == 1, i.e. a rank-1 bias sum_i coef[j,i,0]; it is
    computed on the host and added after the gather.
  - Device: ScalarE tanh (fp16 in/out), VectorE recurrence chain split in
    two independent halves (ic 0-1 / ic 2-3) so q_2 of the first half is
    ready as soon as possible, 112 accumulating TensorE matmuls (fp16,
    N=512, K-contiguous) into 4 PSUM banks, staged to SBUF as fp16 and
    DMA'd out (upcast to f32 on the host).
  - PE warm-up matmuls read the framework's preamble-memset const tiles
    (broadcast AP) so they start right at the end of the preamble with no
    user-side memset dependency -- releases the HAM clock gate early.
  - Input DMA issue is spread over Sync (x0, x1), GpSimd (cf1-ic0, x2,
    x3, cf2..cf7) and Scalar (cf1-ic1:3), in need-by order; coef is
    packed host-side as [d, p, ic, j] so each DMA descriptor moves a
    contiguous 4 KiB row.
  - The last two orders run bank-major (8 matmuls per PSUM bank) so three
    of the four PSUM->SBUF copies + output stores hide under the
    remaining matmuls.

Numerics (vs f64 reference, HW-measured): max err / max|out| ~2.5e-3 --
fp16 matmul inputs, fp32 PSUM accumulation; gate is 2e-2.
"""

import numpy as np

ORDER = 7
ALPHA = 1.0
BETA = 1.0
B_FULL, I_DIM, O_DIM = 4096, 512, 512
N_CORES = 8
BS = B_FULL // N_CORES  # 512 batch rows per core
P = 128                 # SBUF partitions
IC = I_DIM // P         # 4 i-chunks
BT = BS // P            # 4 batch tiles per core


def _recurrence_constants():
    """K1/K3 per reference, rescaled so q_d = t*q_{d-1} - g_d*q_{d-2}."""
    k1 = np.zeros(ORDER + 1, dtype=np.float64)
    k3 = np.zeros(ORDER + 1, dtype=np.float64)
    a, b = ALPHA, BETA
    for i in range(2, ORDER + 1):
        k1[i] = (2 * i + a + b) * (2 * i + a + b - 1) / (2 * i * (i + a + b))
        k3[i] = (
            (i + a - 1) * (i + b - 1) * (2 * i + a + b)
            / (i * (i + a + b) * (2 * i + a + b - 2))
        )
    s = np.zeros(ORDER + 1, dtype=np.float64)
    s[0] = 1.0
    s[1] = 0.5 * (a + b + 2.0)  # p_1 = s_1 * t  (the -(a-b)/2 term is 0)
    for d in range(2, ORDER + 1):
        s[d] = k1[d] * s[d - 1]
    g = np.zeros(ORDER + 1, dtype=np.float64)
    for d in range(2, ORDER + 1):
        g[d] = k3[d] * s[d - 2] / s[d]
    return s, g


_S, _G = _recurrence_constants()

_NC_CACHE = {}


def _build_bass():
    from contextlib import ExitStack
    from concourse import bacc, bass, tile, mybir

    nc = bacc.Bacc(
        "TRN2",
        target_bir_lowering=False,
        debug=False,
        num_devices=1,
    )
    f32 = mybir.dt.float32
    f16 = mybir.dt.float16
    bf16 = mybir.dt.bfloat16

    xT = nc.dram_tensor("xT", [I_DIM, BS], f16, kind="ExternalInput")
    cf = nc.dram_tensor("cf", [ORDER, P, IC, O_DIM], f16, kind="ExternalInput")
    out = nc.dram_tensor("out", [BS, O_DIM], f16, kind="ExternalOutput")

    with tile.TileContext(nc) as tc, ExitStack() as ctx:
        pool = ctx.enter_context(tc.tile_pool(name="main", bufs=1))
        psum = ctx.enter_context(
            tc.tile_pool(name="psum", bufs=1, space=bass.MemorySpace.PSUM)
        )

        # PE warm-up: read the framework's preamble-memset const tile via a
        # broadcast AP -- no user memset, so the first warm-up matmul issues
        # the moment the preamble barrier clears and the HAM clock gate
        # starts its release window as early as possible.
        warm_w = nc.const_aps.tensor(1.0, [P, P], bf16)
        ps_w = psum.tile([P, P], f32, tag="ps_w", name="ps_w")
        N_WARM = 10
        for w in range(N_WARM):
            nc.tensor.matmul(
                ps_w[:], warm_w, warm_w,
                start=(w == 0), stop=(w == N_WARM - 1),
            )

        xt = pool.tile([P, IC, BS], f16, tag="x")
        t = pool.tile([P, IC, BS], f16, tag="t")
        cfs = [None] * (ORDER + 1)
        cf1 = pool.tile([P, IC, O_DIM], f16, tag="cf1", name="cf1")
        cfs[1] = cf1

        # Input DMA issue, in need-by order across the three DMA-capable
        # engines.  Queue (data) order: x0, cf1-ic0, x1, cf1-ic1:3, x2, x3,
        # cf2, .., cf7.
        nc.sync.dma_start(xt[:, 0, :], xT[0 * P:1 * P, :])
        nc.gpsimd.dma_start(cf1[:, 0, :], cf[0, :, 0, :])
        nc.sync.dma_start(xt[:, 1, :], xT[1 * P:2 * P, :])
        nc.scalar.dma_start(cf1[:, 1:, :], cf[0, :, 1:, :])
        nc.gpsimd.dma_start(xt[:, 2, :], xT[2 * P:3 * P, :])
        nc.gpsimd.dma_start(xt[:, 3, :], xT[3 * P:4 * P, :])
        for d in range(2, ORDER + 1):
            c_t = pool.tile([P, IC, O_DIM], f16, tag=f"cf{d}", name=f"cf{d}")
            nc.gpsimd.dma_start(c_t[:], cf[d - 1])
            cfs[d] = c_t

        # tanh: ic0 in two halves so the first d=1 matmuls (b=0,1) unlock
        # as soon as possible after x0 lands.
        Tanh = mybir.ActivationFunctionType.Tanh
        nc.scalar.activation(t[:, 0, 0:256], xt[:, 0, 0:256], Tanh)
        nc.scalar.activation(t[:, 0, 256:512], xt[:, 0, 256:512], Tanh)
        nc.scalar.activation(t[:, 1, :], xt[:, 1, :], Tanh)
        nc.scalar.activation(t[:, 2, :], xt[:, 2, :], Tanh)
        nc.scalar.activation(t[:, 3, :], xt[:, 3, :], Tanh)

        # Recurrence chain, split into two independent halves over the free
        # axis (ic 0-1 and ic 2-3):
        #   q_1 = t; q_2 = t*t - g_2; q_d = t*q_{d-1} - g_d*q_{d-2}
        # The scalar multiply w_d = -g_d * q_{d-2} is a cheap tensor_scalar
        # (4x DVE mode) precomputed one step ahead of the chain.
        q = [None] * (ORDER + 1)
        q[1] = t
        m = [None] * (ORDER + 1)
        w = [None] * (ORDER + 1)
        w[3] = pool.tile([P, IC, BS], f16, tag="w3", name="w3")
        for d in range(2, ORDER + 1):
            m[d] = pool.tile([P, IC, BS], f16, tag=f"m{d}", name=f"m{d}")
            q[d] = pool.tile([P, IC, BS], f16, tag=f"q{d}", name=f"q{d}")
            if d + 2 <= ORDER:
                w[d + 2] = pool.tile(
                    [P, IC, BS], f16, tag=f"w{d+2}", name=f"w{d+2}"
                )
        HALVES = (slice(0, 2), slice(2, 4))
        for h in HALVES:
            nc.vector.tensor_scalar_mul(w[3][:, h, :], t[:, h, :], -float(_G[3]))
            for d in range(2, ORDER + 1):
                nc.vector.tensor_tensor(
                    m[d][:, h, :], t[:, h, :], q[d - 1][:, h, :],
                    mybir.AluOpType.mult,
                )
                if d == 2:
                    # q_0 == 1: tensor_scalar add (DVE 4x mode)
                    nc.vector.tensor_scalar_add(
                        q[d][:, h, :], m[d][:, h, :], -float(_G[d])
                    )
                else:
                    nc.vector.tensor_tensor(
                        q[d][:, h, :], m[d][:, h, :], w[d][:, h, :],
                        mybir.AluOpType.add,
                    )
                if d + 2 <= ORDER:
                    nc.vector.tensor_scalar_mul(
                        w[d + 2][:, h, :], q[d][:, h, :], -float(_G[d + 2])
                    )

        # matmuls: psum[b] += q[d][:, ic, b*128 :+128].T @ cfs[d][:, ic, :]
        # Orders 1..5 ic-major; orders 6-7 bank-major so banks close with an
        # 8-matmul (~1.7us) stagger that hides three of the four PSUM
        # evacuations + output stores.
        ps = [
            psum.tile([P, O_DIM], f32, tag=f"ps{b}", name=f"ps{b}")
            for b in range(BT)
        ]
        for d in range(1, 6):
            for ic in range(IC):
                first = d == 1 and ic == 0
                for b in range(BT):
                    nc.tensor.matmul(
                        ps[b][:],
                        q[d][:, ic, b * P:(b + 1) * P],
                        cfs[d][:, ic, :],
                        start=first,
                        stop=False,
                    )

        ot = pool.tile([P, BT, O_DIM], f16, tag="o")
        for b in range(BT):
            for d in (6, 7):
                for ic in range(IC):
                    nc.tensor.matmul(
                        ps[b][:],
                        q[d][:, ic, b * P:(b + 1) * P],
                        cfs[d][:, ic, :],
                        start=False,
                        stop=(d == 7 and ic == IC - 1),
                    )
            if b % 2 == 0:
                nc.scalar.copy(ot[:, b, :], ps[b][:])
                nc.sync.dma_start(out[b * P:(b + 1) * P, :], ot[:, b, :])
            else:
                nc.vector.tensor_copy(ot[:, b, :], ps[b][:])
                nc.scalar.dma_start(out[b * P:(b + 1) * P, :], ot[:, b, :])

    nc.compile()
    return nc


def _get_nc():
    if "nc" not in _NC_CACHE:
        _NC_CACHE["nc"] = _build_bass()
    return _NC_CACHE["nc"]


def _host_prep(x, coef):
    """Shard + transform inputs. Returns (in_maps, bias)."""
    x = np.asarray(x, dtype=np.float32)
    coef = np.asarray(coef, dtype=np.float32)

    # [d, i, j] with the recurrence scale folded in, orders 1..7
    cf_t = coef.astype(np.float64).transpose(2, 1, 0)  # [8, I, O]
    cf_scaled = (cf_t[1:] * _S[1:, None, None]).astype(np.float16)  # [7, I, O]
    # pack as [d, p, ic, j] (i = ic*128 + p) so each DMA descriptor is a
    # contiguous 4 KiB (ic, j) row per partition
    cf_dev = np.ascontiguousarray(
        cf_scaled.reshape(ORDER, IC, P, O_DIM).transpose(0, 2, 1, 3)
    )
    # d = 0 term: P_0 == 1  ->  bias[j] = sum_i coef[j, i, 0]
    bias = cf_t[0].sum(axis=0)  # [O] f64

    xT = np.ascontiguousarray(x.T.astype(np.float16))  # [I, B] fp16
    in_maps = [
        {"xT": np.ascontiguousarray(xT[:, c * BS:(c + 1) * BS]), "cf": cf_dev}
        for c in range(N_CORES)
    ]
    return in_maps, bias


def kernel(x, coef):
    from concourse.bass_utils import run_bass_kernel_spmd

    nc = _get_nc()
    in_maps, bias = _host_prep(x, coef)
    res = run_bass_kernel_spmd(nc, in_maps, core_ids=list(range(N_CORES)))
    out = np.concatenate(
        [res.results[c]["out"] for c in range(N_CORES)], axis=0
    ).astype(np.float64)
    out += bias[None, :]
    return out.astype(np.float32)
